# revision 23
# baseline (speedup 1.0000x reference)
"""Graphormer attention (N=2048, D=512, H=8 heads of 64) on 8 NeuronCores.

Strategy (tensor-parallel over heads, one head per core), v3:
  - Host slices Q/K/V/O projection weights per head; ships packed p-major
    DRAM blocks so every load is ONE plain 2D DMA (the sync engine pays
    ~600ns per DMA instruction - instruction count matters).
  - The z-bin bias is folded in multiplicatively: host precomputes
    W = exp(z_table[bin(z)]) in the kernel's [key, query] tile layout (fp16).
  - The k-projection bias bk is dropped: it only adds a per-query constant
    to every score row, which cancels exactly in softmax. bq is folded into
    the q evacuation.
  - On device (per core): fused Q^T/K^T projection (Q rows 0-63, K rows
    64-127 of one PSUM block); S^T computed with ROW-TILED matmuls - the
    128x128 PE array splits into two 64-row tiles (T0/T8) running two
    k-tiles concurrently (contraction dim is head_dim=64).
    exp on ScalarE (the critical engine: N^2 elements at 1 elem/lane/cyc),
    P = exp(S) * W on VectorE (fp16 2x mode), O'^T = sum_k V'[k,65] x P
    (65th V column = ones => row 64 of O' is the softmax denominator Z),
    Y^T = Wo_h^T-tiles x O^T in the shadow of the next query-chunk's loop.
  - PSUM budget (8 banks): score pool 2x[128,1024] (4 banks), ot [65,1024]
    (2 banks), scratch 2x[128,512] (2 banks).
  - Host divides each head's partial Y by its Z, sums heads, adds biases.
"""

import numpy as np
import ml_dtypes
from contextlib import ExitStack

import concourse.bass as bass
import concourse.tile as tile
from concourse import bacc, mybir
from concourse import bass_utils

N = 2048
D = 512
H = 8
HD = 64
NUM_Z_BINS = 16
MAX_Z = 5.0
SCALE = HD ** -0.5
NCORES = 8
QL = 1024          # query-chunk length
QC = N // QL       # 2 query chunks
KT = N // 128      # 16 key tiles
NP = KT // 2       # 8 key-tile pairs per query chunk
CH = D // 128      # 4 contraction chunks of the model dim
ROWTILE = True     # S matmul pair on PE tiles T0+T8 (concurrent) vs T0 only

FP32 = mybir.dt.float32
FP16 = mybir.dt.float16
BF16 = mybir.dt.bfloat16
BF16_NP = ml_dtypes.bfloat16
FP16_NP = np.float16

AF = mybir.ActivationFunctionType
OP = mybir.AluOpType

_PROGRAM_CACHE = {}


def _build_program():
    if "nc" in _PROGRAM_CACHE:
        return _PROGRAM_CACHE["nc"]

    nc = bacc.Bacc(
        "TRN2",
        target_bir_lowering=False,
        debug=False,
        enable_asserts=False,
        num_devices=NCORES,
    )

    # packed p-major inputs: every load is one plain 2D DMA
    xjc = nc.dram_tensor("xjc", [4, 128, CH * 512], BF16, kind="ExternalInput").ap()
    wqv = nc.dram_tensor("wqv", [128, CH * 128 + CH * HD], BF16,
                         kind="ExternalInput").ap()
    wo = nc.dram_tensor("wo", [HD, D], FP16, kind="ExternalInput").ap()
    bq = nc.dram_tensor("bq", [HD], FP32, kind="ExternalInput").ap()
    wt = nc.dram_tensor("wt", [QC, NP, 128, 2048], FP16, kind="ExternalInput").ap()

    ypt = nc.dram_tensor("ypt", [QC, 2, 128, 2048], FP16, kind="ExternalOutput").ap()
    zrow = nc.dram_tensor("zrow", [QC, QL], FP16, kind="ExternalOutput").ap()

    with tile.TileContext(nc) as tc:
        with ExitStack() as ctx:
            _emit(ctx, tc, xjc, wqv, wo, bq, wt, ypt, zrow)
    nc.compile()
    _PROGRAM_CACHE["nc"] = nc
    return nc


def _emit(ctx, tc, xjc, wqv, wo, bq, wt, ypt, zrow):
    nc = tc.nc

    singles = ctx.enter_context(tc.tile_pool(name="singles", bufs=1))
    # PSUM: ps_st (score tiles, warmup, QK half-0, tail-Y) 2x[128,1024]
    # = 4 banks; ot [65,1024] = 2 banks; ps_s (scratch: QK half-1,
    # V-proj groups, shadowed-Y) 2x[128,512] = 2 banks.
    ps_st = ctx.enter_context(tc.tile_pool(name="ps_st", bufs=2, space="PSUM"))
    ps_o = ctx.enter_context(tc.tile_pool(name="ps_o", bufs=1, space="PSUM"))
    ps_s = ctx.enter_context(tc.tile_pool(name="ps_s", bufs=2, space="PSUM"))
    wpool = ctx.enter_context(tc.tile_pool(name="wpool", bufs=3))
    epool = ctx.enter_context(tc.tile_pool(name="epool", bufs=5))
    ppool = ctx.enter_context(tc.tile_pool(name="ppool", bufs=5))
    ypool = ctx.enter_context(tc.tile_pool(name="ypool", bufs=2))

    # ---- constants + x^T stream ----------------------------------------
    # ACT exp table preload: tiny dummy exp so the ~1.3us ACT_TABLE_LOAD
    # happens during the input DMA, not before the first real exp.
    scratch = singles.tile([128, 512], BF16)
    nc.vector.memset(scratch, 0.0)
    escratch = singles.tile([128, 1], FP16)
    nc.scalar.activation(escratch, scratch[:, 0:1], AF.Exp)

    # x^T arrives j-major (one DMA per token block j of 512): block j holds
    # [128, c*512+f] = xT[c*128+p, j*512+f], so the QK projection can start
    # after ~1/4 of x^T.
    xT_all = singles.tile([128, 4 * CH * 512], BF16)
    wqv_sb = singles.tile([128, CH * 128 + CH * HD], BF16)
    wo_sb = singles.tile([HD, D], FP16)
    bq_sb = singles.tile([HD, 1], FP32)

    nc.sync.dma_start(out=xT_all[:, 0:2048], in_=xjc[0])
    nc.sync.dma_start(out=wqv_sb, in_=wqv)
    nc.sync.dma_start(out=bq_sb, in_=bq.rearrange("(n a) -> n a", a=1))
    nc.sync.dma_start(out=xT_all[:, 2048:4096], in_=xjc[1])
    nc.sync.dma_start(out=wo_sb, in_=wo)
    nc.sync.dma_start(out=xT_all[:, 4096:6144], in_=xjc[2])
    nc.sync.dma_start(out=xT_all[:, 6144:8192], in_=xjc[3])

    def xs(c, j, off, ln):
        # x^T chunk c, token block j, token offset off within the block
        base = j * 2048 + c * 512 + off
        return xT_all[:, base:base + ln]

    # PE warm-up on never-written scratch: covers the HAM window (~3.4us)
    # while x^T streams in.
    warm = ps_st.tile([128, QL], FP32, tag="st")
    for i in range(12):
        nc.tensor.matmul(warm[:, (i % 2) * 512:(i % 2) * 512 + 512],
                         lhsT=scratch[:, 0:128], rhs=scratch,
                         start=True, stop=True)

    # ---- fused Q^T/K^T projection --------------------------------------
    # One [128,128] weight block: Q^T -> PSUM rows 0-63, K^T -> rows 64-127.
    # qT2/kT2 carry two copies (partitions 0-63 and 64-127) so the row-tiled
    # S matmuls can feed tiles T0 and T8; the dups are SBUF->SBUF DMAs.
    qT2 = singles.tile([128, N], BF16)
    kT2 = singles.tile([128, N], BF16)

    def qk_mms(half, pt, n, cols512):
        j = half * 2 + n
        for c in range(CH):
            nc.tensor.matmul(
                pt[:, cols512],
                lhsT=wqv_sb[:, c * 128:(c + 1) * 128],
                rhs=xs(c, j, 0, 512),
                start=(c == 0),
                stop=(c == CH - 1),
            )

    # half 0: q evac on DVE (bias+scale), k evac on ACT (idle in prologue).
    pth0 = ps_st.tile([128, QL], FP32, tag="st")
    for n in range(2):
        qk_mms(0, pth0, n, slice(n * 512, (n + 1) * 512))
    nc.vector.tensor_scalar(qT2[0:HD, 0:1024], pth0[0:HD, :], bq_sb,
                            SCALE, OP.add, OP.mult)
    nc.scalar.mul(kT2[0:HD, 0:1024], pth0[HD:128, :], 1.0)
    nc.sync.dma_start(out=qT2[HD:128, 0:1024], in_=qT2[0:HD, 0:1024])
    nc.sync.dma_start(out=kT2[HD:128, 0:1024], in_=kT2[0:HD, 0:1024])
    VT_EMITTED = []

    # ---- V' projection: wv-stationary (full PE rate), DMA-transposed ----
    # into a packed 128-stride table: tile t = vpk[:, t*128 : t*128+65],
    # col t*128+64 = 1.0 from the memset (transpose dst offsets must be
    # 128-col aligned - verified on HW).
    v_sb = singles.tile([128, KT * 128], FP16)
    vT_sb = singles.tile([HD, N], FP16)
    nc.vector.memset(v_sb, 1.0)

    def emit_vt(j, engine):
        vp = ps_s.tile([HD, 512], FP32, tag="s", name=f"vp{j}")
        for c in range(CH):
            nc.tensor.matmul(
                vp,
                lhsT=wqv_sb[:, CH * 128 + c * HD:CH * 128 + (c + 1) * HD],
                rhs=xs(c, j, 0, 512),
                start=(c == 0),
                stop=(c == CH - 1),
            )
        dst = slice(j * 512, (j + 1) * 512)
        if engine == "act":
            nc.scalar.copy(vT_sb[:, dst], vp)
        else:
            nc.vector.tensor_copy(vT_sb[:, dst], vp)
        for mi in range(4):
            m = j * 4 + mi
            nc.sync.dma_start(
                out=v_sb[:, m * 128:m * 128 + HD],
                in_=vT_sb[:, m * 128:(m + 1) * 128],
                transpose=True,
            )

    # ---- main loop ------------------------------------------------------
    # Per pair p of k-tiles (2p, 2p+1): row-tiled S (T0/T8, concurrent in
    # the array) -> exp on ACT -> *W on DVE -> PV into ot. ACT is the
    # bottleneck; emission leads by 2 pairs so the PE's in-order queue never
    # makes ACT wait. V-projection groups and the previous chunk's Y blocks
    # ride in the PE's idle slots.
    oT65 = singles.tile([HD + 1, N], FP16)

    def make_emit_s(qc, pending):
        def emit_s(p):
            w_tile = wpool.tile([128, 2048], FP16, tag="w")
            nc.sync.dma_start(out=w_tile, in_=wt[qc, p])
            sts = []
            for i in range(2):
                t = 2 * p + i
                lo, hi = (i * HD, i * HD + HD) if ROWTILE else (0, HD)
                st = ps_st.tile([128, QL], FP32, tag="st", name=f"st{t}")
                for n in range(2):
                    nc.tensor.matmul(
                        st[:, n * 512:(n + 1) * 512],
                        lhsT=kT2[lo:hi, t * 128:(t + 1) * 128],
                        rhs=qT2[lo:hi, qc * QL + n * 512:qc * QL + (n + 1) * 512],
                        start=True, stop=True,
                    )
                sts.append(st)
            pending[p] = (w_tile, sts)
        return emit_s

    def y_pair(qc, mp):
        # Y^T m-blocks 2mp, 2mp+1 of query chunk qc -> one [128,2048] DMA.
        # PSUM via scratch slots; evacuation on DVE (runs in the shadow of
        # the next chunk's loop).
        y_sb = ypool.tile([128, 2048], FP16, tag="ysb")
        for i in range(2):
            m = 2 * mp + i
            for n in range(2):
                yt = ps_s.tile([128, 512], FP32, tag="s")
                nc.tensor.matmul(
                    yt,
                    lhsT=wo_sb[:, m * 128:(m + 1) * 128],
                    rhs=oT65[0:HD, qc * QL + n * 512:qc * QL + (n + 1) * 512],
                    start=True, stop=True,
                )
                nc.vector.tensor_copy(
                    y_sb[:, i * 1024 + n * 512:i * 1024 + (n + 1) * 512], yt)
        nc.sync.dma_start(out=ypt[qc, mp], in_=y_sb)

    y_shadow = []
    for qc in range(QC):
        ot = ps_o.tile([HD + 1, QL], FP32, tag="ot")
        pending = {}
        emit_s = make_emit_s(qc, pending)

        emit_s(0)
        if qc == 0:
            emit_vt(0, "act")
            emit_vt(1, "vec")
            # QK half 1 -> scratch PSUM slots; evacs on DVE (ACT must stay
            # free for the exp stream); k before q (S_B(1) needs the k dup
            # before exp_b(1); q half 1 is only needed for query chunk 1).
            ps_h1 = []
            for n in range(2):
                pth = ps_s.tile([128, 512], FP32, tag="s")
                qk_mms(1, pth, n, slice(0, 512))
                ps_h1.append(pth)
            for n in range(2):
                dst = slice(1024 + n * 512, 1024 + (n + 1) * 512)
                nc.vector.tensor_copy(kT2[0:HD, dst], ps_h1[n][HD:128, :])
            nc.sync.dma_start(out=kT2[HD:128, 1024:2048],
                              in_=kT2[0:HD, 1024:2048])
        emit_s(1)
        if qc == 0:
            for n in range(2):
                dst = slice(1024 + n * 512, 1024 + (n + 1) * 512)
                nc.vector.tensor_scalar(qT2[0:HD, dst], ps_h1[n][0:HD, :],
                                        bq_sb, SCALE, OP.add, OP.mult)
            nc.sync.dma_start(out=qT2[HD:128, 1024:2048],
                              in_=qT2[0:HD, 1024:2048])
            emit_vt(2, "vec")
            emit_vt(3, "vec")

        for p in range(NP):
            if p + 2 < NP:
                emit_s(p + 2)
            if y_shadow and 1 <= p <= 2:
                y_shadow.pop(0)()
            w_tile, sts = pending.pop(p)
            for i in range(2):
                t = 2 * p + i
                st = sts[i]
                e_tile = epool.tile([128, QL], FP16, tag="e")
                nc.scalar.activation(e_tile, st, AF.Exp)
                p_tile = ppool.tile([128, QL], FP16, tag="p")
                nc.vector.tensor_mul(p_tile, e_tile,
                                     w_tile[:, i * QL:(i + 1) * QL])
                for n in range(2):
                    nc.tensor.matmul(
                        ot[:, n * 512:(n + 1) * 512],
                        lhsT=v_sb[:, t * 128:t * 128 + HD + 1],
                        rhs=p_tile[:, n * 512:(n + 1) * 512],
                        start=(t == 0),
                        stop=(t == KT - 1),
                    )

        dst = slice(qc * QL, (qc + 1) * QL)
        if qc == QC - 1:
            # tail: ACT is idle after the last exp; score-pool banks are
            # free for Y PSUM; evacs split across ACT and DVE.
            nc.scalar.copy(oT65[:, dst], ot)
            nc.sync.dma_start(out=zrow[qc].rearrange("(a n) -> a n", a=1),
                              in_=oT65[HD:HD + 1, dst])
            for mp in range(2):
                y_sb = ypool.tile([128, 2048], FP16, tag="ysb")
                for i in range(2):
                    m = 2 * mp + i
                    yt = ps_st.tile([128, QL], FP32, tag="st", name=f"yt{m}")
                    for n in range(2):
                        nc.tensor.matmul(
                            yt[:, n * 512:(n + 1) * 512],
                            lhsT=wo_sb[:, m * 128:(m + 1) * 128],
                            rhs=oT65[0:HD, qc * QL + n * 512:qc * QL + (n + 1) * 512],
                            start=True, stop=True,
                        )
                    if i == 0:
                        nc.scalar.copy(y_sb[:, 0:1024], yt)
                    else:
                        nc.vector.tensor_copy(y_sb[:, 1024:2048], yt)
                nc.sync.dma_start(out=ypt[qc, mp], in_=y_sb)
        else:
            nc.vector.tensor_copy(oT65[:, dst], ot)
            nc.sync.dma_start(out=zrow[qc].rearrange("(a n) -> a n", a=1),
                              in_=oT65[HD:HD + 1, dst])
            y_shadow = [(lambda qc=qc, mp=mp: y_pair(qc, mp)) for mp in range(2)]


def _install_ntff_hook():
    """Recreate the missing ``antenv.axon_hooks`` module so that
    run_bass_kernel_spmd(trace=True) can capture NTFF profiles via the
    libaxon_pjrt.so ctypes hook (see trn_agent_boot.trn_boot)."""
    import sys
    import types

    try:
        import antenv.axon_hooks  # noqa: F401
        return
    except ImportError:
        pass
    import antenv
    from trn_agent_boot.trn_boot import _ntff_profile_via_ctypes

    mod = types.ModuleType("antenv.axon_hooks")
    mod._hook = _ntff_profile_via_ctypes("/opt/axon/libaxon_pjrt.so")
    mod.set_axon_ntff_profile_hook = lambda h: setattr(mod, "_hook", h)
    mod.get_axon_ntff_profile_hook = lambda: mod._hook
    sys.modules["antenv.axon_hooks"] = mod
    antenv.axon_hooks = mod
    # keep profile artifacts local; the sandbox has no bucket access
    bass_utils.upload_artifacts = lambda tmpdir: tmpdir


def kernel(x, z_matrix, Wq, bq, Wk, bk, Wv, bv, Wo, bo, z_table, _trace=False):
    if _trace:
        _install_ntff_hook()
    x = np.ascontiguousarray(np.asarray(x, dtype=np.float32))
    z_matrix = np.asarray(z_matrix, dtype=np.float32)
    Wq = np.asarray(Wq, dtype=np.float32)
    Wk = np.asarray(Wk, dtype=np.float32)
    Wv = np.asarray(Wv, dtype=np.float32)
    Wo = np.asarray(Wo, dtype=np.float32)
    bq = np.asarray(bq, dtype=np.float32)
    bk = np.asarray(bk, dtype=np.float32)
    bv = np.asarray(bv, dtype=np.float32)
    bo = np.asarray(bo, dtype=np.float32)
    z_table = np.asarray(z_table, dtype=np.float32)

    nc = _build_program()

    xT = x.T  # [D, N]
    # xjc[j, p, c*512 + f] = xT[c*128 + p, j*512 + f]
    xjc = np.ascontiguousarray(
        xT.reshape(CH, 128, 4, 512).transpose(2, 1, 0, 3).reshape(4, 128, CH * 512)
    ).astype(BF16_NP)
    binsT = np.clip(
        np.floor(z_matrix.T / MAX_Z * NUM_Z_BINS).astype(np.int32), 0, NUM_Z_BINS - 1
    )
    exp_tab = np.exp(z_table)  # [16, H] fp32

    in_maps = []
    for h in range(NCORES):
        sl = slice(h * HD, (h + 1) * HD)
        w_full = exp_tab[:, h][binsT].astype(FP16_NP)  # [key, query]
        # wt[qc, pr, p, i*1024 + f] = w_full[(2pr+i)*128 + p, qc*1024 + f]
        wt_h = np.ascontiguousarray(
            w_full.reshape(NP, 2, 128, QC, QL).transpose(3, 0, 2, 1, 4)
            .reshape(QC, NP, 128, 2048))
        # wqv[p, c*128+m] = Wqk[c*128+p, m]; wqv[p, 512 + c*64+m] = Wv[c*128+p, m]
        wqk_h = np.concatenate([Wq[:, sl], Wk[:, sl]], axis=1)  # [D, 128]
        wqv_h = np.concatenate([
            wqk_h.reshape(CH, 128, 128).transpose(1, 0, 2).reshape(128, CH * 128),
            Wv[:, sl].reshape(CH, 128, HD).transpose(1, 0, 2).reshape(128, CH * HD),
        ], axis=1)
        in_maps.append({
            "xjc": xjc,
            "wqv": np.ascontiguousarray(wqv_h).astype(BF16_NP),
            "wo": np.ascontiguousarray(Wo[sl, :]).astype(FP16_NP),
            "bq": np.ascontiguousarray(bq[sl]),
            "wt": wt_h,
        })

    res = bass_utils.run_bass_kernel_spmd(
        nc, in_maps, core_ids=list(range(NCORES)), trace=_trace,
    )

    acc = np.zeros((D, N), dtype=np.float64)
    for h in range(NCORES):
        ypt_h = res.results[h]["ypt"].astype(np.float64)  # [QC, 2, 128, 2048]
        # ypT[(2mp+i)*128 + p, qc*1024 + f] = ypt[qc, mp, p, i*1024 + f]
        ypT_h = (ypt_h.reshape(QC, 2, 128, 2, QL).transpose(1, 3, 2, 0, 4)
                 .reshape(D, N))
        z_h = res.results[h]["zrow"].astype(np.float64).reshape(N)
        acc += ypT_h / z_h[None, :]
    out = acc.T + (bv @ Wo)[None, :] + bo[None, :]
    out_f32 = out.astype(np.float32)
    if _trace:
        return out_f32, res
    return out_f32
